# revision 26
# baseline (speedup 1.0000x reference)
"""GQA attention layer (B=1, S=2048, H=4096, 32 Q heads / 8 KV heads, RoPE,
causal) on 8 trn2 NeuronCores, head-parallel: core i owns Q heads 4i..4i+3
and KV head i (column-parallel wq/wk/wv, row-parallel wo; partial outputs
summed on host)."""

import sys

if "/opt/trn_rl_repo" not in sys.path:
    sys.path.insert(0, "/opt/trn_rl_repo")

import contextlib

import numpy as np

import concourse.bacc as bacc
import concourse.mybir as mybir
import concourse.tile as tile
from concourse.bass_isa import ReduceOp
from concourse.bass_utils import run_bass_kernel_spmd

F32 = mybir.dt.float32
F32R = mybir.dt.float32r
EXP = mybir.ActivationFunctionType.Exp

HIDDEN = 4096
S = 2048
HD = 128
NCORES = 8
QH = 4            # Q heads per core
KT_H = HIDDEN // 128   # 32 hidden k-tiles
NQC = S // 512         # 4 query chunks of 512
NST = S // 128         # 16 sequence tiles of 128

_CACHE = {}


def _build(reps=1, loop_n=0, phases=(1, 2, 3), p1_mode=0, p2_mode=0):
    nc = bacc.Bacc(None, target_bir_lowering=False)

    xt = nc.dram_tensor("xt", [HIDDEN, S], F32, kind="ExternalInput")
    wq = nc.dram_tensor("wq", [HIDDEN, QH * HD], F32, kind="ExternalInput")
    wk = nc.dram_tensor("wk", [HIDDEN, HD], F32, kind="ExternalInput")
    wv = nc.dram_tensor("wv", [HIDDEN, HD], F32, kind="ExternalInput")
    wo = nc.dram_tensor("wo", [QH * HD, HIDDEN], F32, kind="ExternalInput")
    cosd = nc.dram_tensor("cosd", [64, S], F32, kind="ExternalInput")
    sind = nc.dram_tensor("sind", [64, S], F32, kind="ExternalInput")
    maskd = [nc.dram_tensor(f"mask{o}", [128, 512], F32, kind="ExternalInput")
             for o in range(4)]
    onesc = nc.dram_tensor("onesc", [128, 1], F32, kind="ExternalInput")
    onesr = nc.dram_tensor("onesr", [1, 128], F32, kind="ExternalInput")
    ident = nc.dram_tensor("ident", [128, 128], F32, kind="ExternalInput")
    out = nc.dram_tensor("out", [HIDDEN, S], F32, kind="ExternalOutput")

    with tile.TileContext(nc) as tc:
      with (tc.For_i(0, loop_n, 1) if loop_n else contextlib.nullcontext()):
       for _rep in range(reps):
        with tc.tile_pool(name="persist", bufs=1) as persist:
            qt_sb = [persist.tile([128, S], F32R, tag=f"qt{h}", name=f"qt{h}") for h in range(QH)]
            kt_sb = persist.tile([128, S], F32R, tag="kt", name="kt")
            v_sb = [persist.tile([128, 128], F32R, tag=f"v{i}", name=f"v{i}") for i in range(NST)]
            cs_sb = persist.tile([128, S], F32, tag="cs", name="cs")
            cos_sb = cs_sb[0:64, :]
            sin_sb = cs_sb[64:128, :]
            onesc_sb = persist.tile([128, 1], F32R, tag="onesc", name="onesc")
            onesr_sb = persist.tile([1, 128], F32R, tag="onesr", name="onesr")
            ident_sb = persist.tile([128, 128], F32, tag="ident", name="ident")

            nc.sync.dma_start(out=cs_sb[0:64, :], in_=cosd[:, :])
            nc.sync.dma_start(out=cs_sb[64:128, :], in_=sind[:, :])
            nc.sync.dma_start(out=onesc_sb, in_=onesc[:, :].bitcast(F32R))
            nc.sync.dma_start(out=onesr_sb, in_=onesr[:, :].bitcast(F32R))
            nc.sync.dma_start(out=ident_sb, in_=ident[:, :])

            # ---------------- Phase 1: projections + RoPE -----------------
            with tc.tile_pool(name="p1xt", bufs=4) as p_xt, \
                 tc.tile_pool(name="p1wq", bufs=1) as p_wq, \
                 tc.tile_pool(name="p1wkv", bufs=1) as p_wkv, \
                 tc.tile_pool(name="p1rope", bufs=1) as p_rope, \
                 tc.tile_pool(name="p1vt", bufs=2) as p_vt, \
                 tc.tile_pool(name="ps1", bufs=1, space="PSUM") as ps1, \
                 tc.tile_pool(name="pst", bufs=2, space="PSUM") as pst:
                wk_t = [None] * KT_H
                wv_t = [None] * KT_H
                wq_res = []
                for k4 in range(KT_H // 4):
                    wq8 = p_wq.tile([128, 4, 512], F32R, tag=f"wq8_{k4}", name=f"wq8_{k4}")
                    srcw = wq[k4 * 512:(k4 + 1) * 512, :].bitcast(F32R)
                    nc.scalar.dma_start(out=wq8,
                                        in_=srcw.rearrange("(j p) c -> p j c", p=128))
                    wq_res.append(wq8)
                if p1_mode == 1:
                    xt_hoist = p_xt.tile([128, 512], F32R, tag="xt", name="xt_t")
                    nc.sync.dma_start(out=xt_hoist, in_=xt[0:128, 0:512].bitcast(F32R))
                    wq_hoist = p_wq.tile([128, 512], F32R, tag="wq", name="wq_t")
                    nc.sync.dma_start(out=wq_hoist, in_=wq[0:128, :].bitcast(F32R))
                for qc in range(NQC):
                    qs = slice(qc * 512, (qc + 1) * 512)
                    psq = [ps1.tile([128, 512], F32, tag=f"q{d}", name=f"psq{d}") for d in range(QH)]
                    psk = ps1.tile([128, 512], F32, tag="k", name="psk")
                    psv = ps1.tile([128, 512], F32, tag="v", name="psv")
                    xt4 = wq4 = None
                    for k in range(KT_H):
                        ks = slice(k * 128, (k + 1) * 128)
                        j = k % 4
                        if p1_mode == 1:
                            xt_t = xt_hoist
                            wq_t = wq_hoist
                        else:
                            if k % 2 == 0:
                                # batched loads: 2 k-slices per DMA (512 KB each)
                                k2 = k // 2
                                xt4 = p_xt.tile([128, 2, 512], F32R, tag="xt", name="xt4")
                                src = xt[k2 * 256:(k2 + 1) * 256, qs].bitcast(F32R)
                                nc.sync.dma_start(
                                    out=xt4,
                                    in_=src.rearrange("(j p) c -> p j c", p=128))
                            xt_t = xt4[:, k % 2, :]
                            wq_t = wq_res[k // 4][:, k % 4, :]
                        if p1_mode == 2:
                            continue
                        if qc == 0:
                            wk_t[k] = p_wkv.tile([128, 128], F32R, tag=f"wk{k}", name=f"wk{k}")
                            nc.sync.dma_start(out=wk_t[k], in_=wk[ks, :].bitcast(F32R))
                            wv_t[k] = p_wkv.tile([128, 128], F32R, tag=f"wv{k}", name=f"wv{k}")
                            nc.sync.dma_start(out=wv_t[k], in_=wv[ks, :].bitcast(F32R))
                        st = (k == 0)
                        sp = (k == KT_H - 1)
                        for d in range(QH):
                            nc.tensor.matmul(psq[d][:, :], wq_t[:, d * 128:(d + 1) * 128],
                                             xt_t[:, :], start=st, stop=sp)
                        nc.tensor.matmul(psk[:, :], wk_t[k][:, :], xt_t[:, :],
                                         start=st, stop=sp)
                        nc.tensor.matmul(psv[:, :], wv_t[k][:, :], xt_t[:, :],
                                         start=st, stop=sp)

                    if p1_mode == 2:
                        continue
                    # RoPE: head_dim deinterleaved (evens in partitions 0:64,
                    # odds in 64:128) so the rotation uses contiguous halves.
                    ct = cos_sb[:, qs]
                    snt = sin_sb[:, qs]
                    for d in range(QH + 1):
                        src = psk if d == QH else psq[d]
                        dst = kt_sb if d == QH else qt_sb[d]
                        ec = p_rope.tile([64, 512], F32, tag="ec", name="ec")
                        os_ = p_rope.tile([64, 512], F32, tag="os", name="os_")
                        es = p_rope.tile([64, 512], F32, tag="es", name="es")
                        oc = p_rope.tile([64, 512], F32, tag="oc", name="oc")
                        nc.vector.tensor_mul(ec, src[0:64, :], ct)
                        nc.vector.tensor_mul(os_, src[64:128, :], snt)
                        nc.vector.tensor_sub(dst[0:64, qs], ec, os_)
                        nc.vector.tensor_mul(es, src[0:64, :], snt)
                        nc.vector.tensor_mul(oc, src[64:128, :], ct)
                        nc.vector.tensor_add(dst[64:128, qs], es, oc)

                    # V: copy PSUM->SBUF then PE-transpose 128x128 chunks
                    vt_t = p_vt.tile([128, 512], F32, tag="vt", name="vt_t")
                    nc.scalar.copy(vt_t[:, :], psv[:, :])
                    for j in range(4):
                        ptr = pst.tile([128, 128], F32, tag="pst", name="ptr")
                        nc.tensor.transpose(ptr[:, :], vt_t[:, j * 128:(j + 1) * 128],
                                            ident_sb[:, :])
                        nc.scalar.copy(v_sb[qc * 4 + j][:, :], ptr[:, :])

            # ---------------- Phases 2+3 ------------------------
            with tc.tile_pool(name="p23", bufs=1) as p23:
             aot_sb = [p23.tile([128, S], F32R, tag=f"aot{h}", name=f"aot{h}")
                       for h in range(QH)]
             mask_sb = [p23.tile([128, 512], F32R, tag=f"mask{o}", name=f"mask{o}")
                        for o in range(4)]
             for o in range(4):
                 nc.scalar.dma_start(out=mask_sb[o], in_=maskd[o][:, :].bitcast(F32R))
             # ---------------- Phase 2: attention ------------------------
             if 2 in phases:
              with tc.tile_pool(name="p2p", bufs=3) as p_p, \
                                  tc.tile_pool(name="p2dacc", bufs=2) as p_dacc, \
                 tc.tile_pool(name="p2recip", bufs=2) as p_recip, \
                 tc.tile_pool(name="p2r", bufs=2) as p_r, \
                 tc.tile_pool(name="ps2s", bufs=2, space="PSUM") as ps2s, \
                 tc.tile_pool(name="ps2o", bufs=2, space="PSUM") as ps2o, \
                 tc.tile_pool(name="ps2d", bufs=1, space="PSUM") as ps2d, \
                 tc.tile_pool(name="ps2r", bufs=1, space="PSUM") as ps2r:
                kstate = {}

                def produce(h, qc, g):
                    if g == 0:
                        kstate[(h, qc)] = (
                            ps2o.tile([128, 512], F32, tag="o", name="ps_o"),
                            p_dacc.tile([128, 1024], F32R, tag="dacc", name="dacc"),
                            {},
                        )
                    ps_o, dacc, handles = kstate[(h, qc)]
                    qs = slice(qc * 512, (qc + 1) * 512)
                    ps_s = ps2s.tile([128, 1024], F32, tag="s", name="ps_s")
                    for u in range(2):
                        j = 2 * g + u
                        nc.tensor.matmul(ps_s[:, u * 512:(u + 1) * 512],
                                         kt_sb[:, j * 128:(j + 1) * 128],
                                         qt_sb[h][:, qs], start=True, stop=True)
                    handles[("s", g)] = ps_s

                def expmask(h, qc, g):
                    ps_o, dacc, handles = kstate[(h, qc)]
                    ps_s = handles.pop(("s", g))
                    p_t = p_p.tile([128, 1024], F32R, tag="p", name="p_t")
                    nc.scalar.activation(p_t[:, :], ps_s[:, :], EXP)
                    if p2_mode == 0:
                        for u in range(2):
                            o = 2 * g + u - 4 * qc
                            if o >= 0:
                                nc.vector.tensor_mul(p_t[:, u * 512:(u + 1) * 512],
                                                     p_t[:, u * 512:(u + 1) * 512],
                                                     mask_sb[o][:, :])
                        if g == 0:
                            nc.vector.tensor_copy(dacc[:, :], p_t[:, :])
                        else:
                            nc.vector.tensor_add(dacc[:, :], dacc[:, :], p_t[:, :])
                    handles[("p", g)] = p_t

                def pv(h, qc, g, n_kt):
                    ps_o, dacc, handles = kstate[(h, qc)]
                    p_t = handles.pop(("p", g))
                    for u in range(2):
                        j = 2 * g + u
                        nc.tensor.matmul(ps_o[:, :], v_sb[j][:, :],
                                         p_t[:, u * 512:(u + 1) * 512],
                                         start=(j == 0), stop=(j == n_kt - 1))

                def emit_epilogue(h, qc):
                    ps_o, dacc, handles = kstate.pop((h, qc))
                    qs = slice(qc * 512, (qc + 1) * 512)
                    if p2_mode == 1:
                        nc.scalar.copy(aot_sb[h][:, qs], ps_o[:, :])
                        return
                    ps_d = ps2d.tile([1, 512], F32, tag="d", name="ps_d")
                    nc.tensor.matmul(ps_d[:, :], onesc_sb[:, :], dacc[:, 0:512],
                                     start=True, stop=False)
                    nc.tensor.matmul(ps_d[:, :], onesc_sb[:, :], dacc[:, 512:1024],
                                     start=False, stop=True)
                    recip = p_recip.tile([1, 512], F32R, tag="recip", name="recip")
                    with nc.allow_low_precision(reason="f32r softmax denom reciprocal"):
                        nc.vector.reciprocal(recip[:, :], ps_d[:, :])
                    ps_rf = ps2r.tile([128, 512], F32, tag="rf", name="ps_rf")
                    nc.tensor.matmul(ps_rf[:, :], onesr_sb[:, :], recip[:, :],
                                     start=True, stop=True)
                    r_sb = p_r.tile([128, 512], F32, tag="r", name="r_sb")
                    nc.scalar.copy(r_sb[:, :], ps_rf[:, :])
                    nc.vector.tensor_mul(aot_sb[h][:, qs], ps_o[:, :], r_sb[:, :])

                units = []
                for h in range(QH):
                    for qc in range(NQC):
                        n_g = (4 * qc + 4) // 2
                        for g in range(n_g):
                            units.append((h, qc, g, 4 * qc + 4, g == n_g - 1))
                nu = len(units)
                pending = []  # (emit_after_idx, h, qc)
                for idx in range(nu + 2):
                    if idx < nu:
                        produce(*units[idx][:3])
                    if idx >= 1 and idx - 1 < nu:
                        expmask(*units[idx - 1][:3])
                    if idx >= 2:
                        h, qc, g, n_kt, last = units[idx - 2]
                        pv(h, qc, g, n_kt)
                        if last:
                            pending.append((idx + 4, h, qc))
                    while pending and pending[0][0] <= idx:
                        _, eh, eqc = pending.pop(0)
                        emit_epilogue(eh, eqc)
                for _, eh, eqc in pending:
                    emit_epilogue(eh, eqc)

             # ---------------- Phase 3: output projection ------------------
             if 3 in phases:
              with tc.tile_pool(name="p3wo", bufs=2) as p_wo, \
                 tc.tile_pool(name="p3ob", bufs=3) as p_ob, \
                 tc.tile_pool(name="ps3", bufs=3, space="PSUM") as ps3:
                for nc4 in range(8):
                    ns = slice(nc4 * 512, (nc4 + 1) * 512)
                    wo_t = []
                    for c in range(4):
                        t = p_wo.tile([128, 512], F32R, tag=f"wo{c}", name=f"wo{c}")
                        nc.sync.dma_start(out=t, in_=wo[c * 128:(c + 1) * 128, ns].bitcast(F32R))
                        wo_t.append(t)
                    for nt in range(4):
                        for sc2 in range(NQC // 2):
                            ps = ps3.tile([128, 1024], F32, tag="ps", name="ps")
                            for u in range(2):
                                ss = slice((2 * sc2 + u) * 512, (2 * sc2 + u + 1) * 512)
                                for c in range(4):
                                    nc.tensor.matmul(ps[:, u * 512:(u + 1) * 512],
                                                     wo_t[c][:, nt * 128:(nt + 1) * 128],
                                                     aot_sb[c][:, ss],
                                                     start=(c == 0), stop=(c == 3))
                            ob = p_ob.tile([128, 1024], F32, tag="ob", name="ob")
                            nc.scalar.copy(ob[:, :], ps[:, :])
                            nc.sync.dma_start(
                                out=out[(nc4 * 4 + nt) * 128:(nc4 * 4 + nt + 1) * 128,
                                        2 * sc2 * 512:(2 * sc2 + 2) * 512],
                                in_=ob[:, :])

             if 2 not in phases:
                nc.sync.dma_start(out=out[0:128, 0:512], in_=qt_sb[0][:, 0:512].bitcast(F32))
             if 3 not in phases and 2 in phases:
                for _h in range(QH):
                    nc.sync.dma_start(out=out[_h * 128:(_h + 1) * 128, :],
                                      in_=aot_sb[_h][:, :].bitcast(F32))
    nc.compile()
    return nc


def get_nc():
    if "nc" not in _CACHE:
        _CACHE["nc"] = _build()
    return _CACHE["nc"]


def prep_in_maps(hidden_states, attention_mask, position_ids, wq, wk, wv, wo):
    hs = np.asarray(hidden_states, dtype=np.float32)
    pos = np.asarray(position_ids)
    wq = np.asarray(wq, dtype=np.float32)
    wk = np.asarray(wk, dtype=np.float32)
    wv = np.asarray(wv, dtype=np.float32)
    wo = np.asarray(wo, dtype=np.float32)

    xt = np.ascontiguousarray(hs[0].T)  # [HIDDEN, S]

    inv = 1.0 / (10000.0 ** (np.arange(0, HD, 2, dtype=np.float64) / HD))  # [64]
    freqs = inv[:, None] * pos[0].astype(np.float64)[None, :]  # [64, S]
    cos = np.cos(freqs).astype(np.float32)
    sin = np.sin(freqs).astype(np.float32)

    perm = np.concatenate([np.arange(0, HD, 2), np.arange(1, HD, 2)])
    scale = np.float32(1.0 / np.sqrt(HD))

    kk = np.arange(128)[:, None]
    qq = np.arange(512)[None, :]
    masks = [np.ascontiguousarray((kk + 128 * o <= qq).astype(np.float32))
             for o in range(4)]
    onesc = np.ones((128, 1), np.float32)
    onesr = np.ones((1, 128), np.float32)
    ident = np.eye(128, dtype=np.float32)

    in_maps = []
    for i in range(NCORES):
        wq_i = wq[:, i * 512:(i + 1) * 512].reshape(HIDDEN, QH, HD)[:, :, perm]
        wq_i = np.ascontiguousarray(wq_i.reshape(HIDDEN, QH * HD) * scale)
        wk_i = np.ascontiguousarray(wk[:, i * HD:(i + 1) * HD][:, perm])
        wv_i = np.ascontiguousarray(wv[:, i * HD:(i + 1) * HD])
        wo_i = np.ascontiguousarray(wo[i * 512:(i + 1) * 512, :])
        in_maps.append({
            "xt": xt, "wq": wq_i, "wk": wk_i, "wv": wv_i, "wo": wo_i,
            "cosd": cos, "sind": sin,
            "mask0": masks[0], "mask1": masks[1], "mask2": masks[2], "mask3": masks[3],
            "onesc": onesc, "onesr": onesr, "ident": ident,
        })
    return in_maps


def kernel(hidden_states, attention_mask, position_ids, wq, wk, wv, wo):
    in_maps = prep_in_maps(hidden_states, attention_mask, position_ids,
                           wq, wk, wv, wo)
    nc = get_nc()
    res = run_bass_kernel_spmd(nc, in_maps, core_ids=list(range(NCORES)))
    total = res.results[0]["out"].astype(np.float32)
    for i in range(1, NCORES):
        total = total + res.results[i]["out"]
    return np.ascontiguousarray(total.T).reshape(1, S, HIDDEN)
